# revision 3
# baseline (speedup 1.0000x reference)
"""4D Conv-MLP (conv3^4 -> ReLU -> conv3^4) on 8 Trainium2 NeuronCores.

Sharding: core = b*4 + j (batch b in {0,1}, H-slab j in {0..3}, 8 output rows
each). Conv1 is recomputed on a 1-row h halo (10 h rows from 12 x rows), so no
cross-core communication. One SPMD program; boundary behavior is data-driven
(host-zeroed x halos + h halo-row masks).

Winograd F(4,3) along W on BOTH convs (2x fewer multiplies vs direct): the 3
kw taps become 6 pointwise components m0..m5 evaluated at 8 stride-4 w-tiles;
y = A^T m (4 outputs per tile). T/D/H taps (kt, ku, kv) stay direct.

  - x~ (input transform, 6 comps) built on host; comps packed in PAIRS
    (1,2),(3,4),(0,5) on the 128 partitions (64 ch each half) so two K=64
    matmuls co-stream via tile_position (0,0)/(64,0).
  - d runs are uniform via ZERO-PADDED d planes (18 = 1+16+1): conv1 runs
    nd=6,6,4 (N<=480), conv2 nd=8,8 (N=512); all 3 ku taps always valid
    (pad-plane taps multiply zeros).
  - conv1 PSUM: 6 chains/run, one bank each; pair (1,2) double-buffered,
    (3,4)/(0,5) single (8 banks exactly). Inverse transform pipelines at
    pair granularity: a=m1+m2, b=m1-m2 | c=m3+m4, d=m3-m4, e=a+c,
    y1=2d+b, y2=4c+a, y3p=8d+b | y0=m0+e, y3=y3p+m5 (scalar_tensor_tensor
    fused ops); ReLU+b1 in-place on Scalar; halo rows masked; h~ (conv2's
    F(4,3) forward transform) built with 12 fused DVE ops from the 4
    phase planes (pad cols give the k+-1 shifts).
  - conv2: K=128 matmuls, comps col-split co-streamed via tile_position
    (0,0)/(0,64) into one bank per pair; same 10-op inverse; bias via
    Scalar; one DMA per run to y (4 phase planes, host interleaves).
All matmul operands fp16, PSUM accumulation fp32. PE is stream-bound:
~595k column-cycles @2.4GHz ~ 248us.
"""

import numpy as np

B, C_IN, C_HID, C_OUT = 2, 64, 128, 64
T, D, H, W = 4, 16, 32, 32
NCORES, NJ = 8, 4
SH = H // NJ          # 8 out rows per slab
XH = SH + 4           # 12 x rows per slab
HR = SH + 2           # 10 h rows per slab (1-row halo each side)
KW = W // 4           # 8 winograd w-tiles
DP = D + 2            # zero-padded d planes
XPL = D * XH * KW     # x~ real planes per t per comp-pair = 1536
PAIRS = ((1, 2), (3, 4), (0, 5))
RUNS1 = ((0, 6), (6, 6), (12, 4))
RUNS2 = ((0, 8), (8, 8))

# F(4,3) transform matrices (float64 host math)
_BT = np.array([[4, 0, -5, 0, 1, 0], [0, -4, -4, 1, 1, 0], [0, 4, -4, -1, 1, 0],
                [0, -2, -1, 2, 1, 0], [0, 2, -1, -2, 1, 0], [0, 4, 0, -5, 0, 1]],
               np.float64)
_G = np.array([[1 / 4, 0, 0], [-1 / 6, -1 / 6, -1 / 6], [-1 / 6, 1 / 6, -1 / 6],
               [1 / 24, 1 / 12, 1 / 6], [1 / 24, -1 / 12, 1 / 6], [0, 0, 1]],
              np.float64)

_cache = {}


def _t_taps(t):
    return [kt for kt in range(3) if 0 <= t + kt - 1 < T]


def _g27(kt, ku, kv):
    return (kt * 3 + ku) * 3 + kv


def _make_host_arrays(x, w1, b1, w2, b2):
    x = np.asarray(x, np.float32)
    XPs, AUXs = [], []
    for core in range(NCORES):
        b, j = divmod(core, NJ)
        h0 = SH * j
        slab = np.zeros((C_IN, T, D, XH, W + 6), np.float32)
        lo, hi = h0 - 2, h0 + 10
        slo, shi = max(lo, 0), min(hi, H)
        slab[:, :, :, slo - lo:shi - lo, 1:33] = x[b, :, :, :, slo:shi, :]
        # F(4,3) input transform along W: tile k reads padded cols 4k..4k+5
        win = np.stack([slab[..., 4 * k:4 * k + 6] for k in range(KW)], -2)
        xt = (win @ _BT.T).astype(np.float16)    # [64, T, D, XH, KW, 6]
        ps = []
        for ca, cb in PAIRS:
            ps.append(np.concatenate([xt[..., ca], xt[..., cb]], 0)
                      .reshape(128, T, XPL))
        XPs.append(ps)
        a = np.zeros((128, 4), np.float32)
        a[:, 0] = np.asarray(b1, np.float32)
        a[:, 1] = 0.0 if j == 0 else 1.0
        a[:, 2] = 0.0 if j == NJ - 1 else 1.0
        a[0:64, 3] = np.asarray(b2, np.float32)
        AUXs.append(a)

    w1 = np.asarray(w1, np.float64)   # [128, 64, 3,3,3,3]
    w2 = np.asarray(w2, np.float64)   # [64, 128, 3,3,3,3]
    w1t = np.einsum('oitduv,cv->oitduc', w1, _G)   # [128, 64, 3,3,3, 6]
    w2t = np.einsum('oitduv,cv->oitduc', w2, _G)   # [64, 128, 3,3,3, 6]
    W1P = np.zeros((128, 3, 27, 128), np.float16)
    W2P = np.zeros((128, 27, 6, 64), np.float16)
    for kt in range(3):
        for ku in range(3):
            for kv in range(3):
                g = _g27(kt, ku, kv)
                for p, (ca, cb) in enumerate(PAIRS):
                    W1P[0:64, p, g, :] = w1t[:, :, kt, ku, kv, ca].T
                    W1P[64:128, p, g, :] = w1t[:, :, kt, ku, kv, cb].T
                for c in range(6):
                    W2P[:, g, c, :] = w2t[:, :, kt, ku, kv, c].T
    return dict(XP=XPs, AUX=AUXs,
                W1=W1P.reshape(128, 3 * 27 * 128), W2=W2P.reshape(128, 27 * 6 * 64))


def _build_module():
    import concourse.tile as tile
    from concourse import bacc, mybir

    fp16 = mybir.dt.float16
    fp32 = mybir.dt.float32
    RELU = mybir.ActivationFunctionType.Relu
    IDENT = mybir.ActivationFunctionType.Identity
    MULT = mybir.AluOpType.mult
    ADD = mybir.AluOpType.add
    SUB = mybir.AluOpType.subtract

    nc = bacc.Bacc("TRN2", target_bir_lowering=False, debug=False, num_devices=1)
    xp_d = [nc.dram_tensor(f"xp{p}", [128, T, XPL], fp16, kind="ExternalInput")
            for p in range(3)]
    w1_d = nc.dram_tensor("w1", [128, 3 * 27 * 128], fp16, kind="ExternalInput")
    w2_d = nc.dram_tensor("w2", [128, 27 * 6 * 64], fp16, kind="ExternalInput")
    # aux: col0=b1, col1=mt, col2=mb, col3[:64]=b2
    aux_d = nc.dram_tensor("aux", [128, 4], fp32, kind="ExternalInput")
    # 4 phase planes (w%4) stored separately; host interleaves
    y_d = nc.dram_tensor("y", [64, T, 4, D * SH * KW], fp16,
                         kind="ExternalOutput")

    with tile.TileContext(nc) as tc:
        with (
            tc.tile_pool(name="xw", bufs=1) as xw,
            tc.tile_pool(name="st", bufs=2) as stp,
            tc.tile_pool(name="pp", bufs=1, space="PSUM") as pp,
        ):
            xP0 = xw.tile([128, T, DP, XH, KW], fp16)
            xP1 = xw.tile([128, T, DP, XH, KW], fp16)
            xP2 = xw.tile([128, T, DP, XH, KW], fp16)
            xPs = (xP0, xP1, xP2)
            w1 = xw.tile([128, 3, 27, 128], fp16)
            w2 = xw.tile([128, 27, 6, 64], fp16)
            hT = xw.tile([128, T, 6, DP, HR, KW], fp16)
            aux = xw.tile([128, 4], fp32)

            # zero the d pad planes (never DMA'd / written by conv1)
            for xp in xPs:
                nc.vector.memset(xp[:, :, 0, :, :], 0.0)
                nc.vector.memset(xp[:, :, DP - 1, :, :], 0.0)
            nc.vector.memset(hT[:, :, :, 0, :, :], 0.0)
            nc.vector.memset(hT[:, :, :, DP - 1, :, :], 0.0)

            # DMA order: unblock conv1 t=0 run(0,6) fast — its tap order
            # is g 12..17, 9..11, 21..26, 18..20 on d planes 0..7. Input
            # transfers alternate between the two HW DGE queues (qSP via
            # nc.sync, qAct via nc.scalar) — each queue drains serially.
            def idma(dst, srcv):
                nc.sync.dma_start(dst, srcv)

            def w1chunk(p, glo, ghi):
                idma(w1[:, p, glo:ghi, :],
                     w1_d.ap()[:, p * 3456 + glo * 128:p * 3456 + ghi * 128])

            for p in range(3):
                if p == 0:
                    w1chunk(0, 12, 15)
                    w1chunk(0, 15, 18)
                    idma(xPs[0][:, 0, 1:5, :, :], xp_d[0].ap()[:, 0, 0:384])
                    idma(xPs[0][:, 0, 5:9, :, :], xp_d[0].ap()[:, 0, 384:768])
                    w1chunk(0, 9, 12)
                    idma(xPs[0][:, 1, 1:5, :, :], xp_d[0].ap()[:, 1, 0:384])
                    idma(xPs[0][:, 1, 5:9, :, :], xp_d[0].ap()[:, 1, 384:768])
                    w1chunk(0, 18, 27)
                else:
                    w1chunk(p, 9, 27)
                    for t in (0, 1):
                        idma(xPs[p][:, t, 1:9, :, :], xp_d[p].ap()[:, t, 0:768])
            idma(aux[:, :], aux_d.ap())
            for p in range(3):
                for t in (0, 1):
                    idma(xPs[p][:, t, 9:DP - 1, :, :],
                         xp_d[p].ap()[:, t, 768:1536])
                w1chunk(p, 0, 9)
            for p in range(3):
                for t in (2, 3):
                    idma(xPs[p][:, t, 1:DP - 1, :, :], xp_d[p].ap()[:, t, :])
            idma(w2[:, :, :, :], w2_d.ap())
            b1 = aux[:, 0:1]
            mt = aux[:, 1:2]
            mb = aux[:, 2:3]
            b2 = aux[0:64, 3:4]

            # prime the phase-plane staging pads (cols 0 and 9 stay zero)
            for _ in range(2):
                ph = stp.tile([128, 4, 6, HR, KW + 2], fp16)
                nc.vector.memset(ph[:, :, :, :, :], 0.0)

            # ---- conv1: F(4,3) comps, K=64 row-split co-stream ----
            def conv1_t(t, runs):
                kts = _t_taps(t)
                for dlo, nd in runs:
                    # pad-plane taps stream partial N (trimmed rows); first
                    # tap must be untrimmed so start=True clears the full bank
                    kuo = (1, 2, 0) if dlo == 0 else (0, 1, 2)
                    taps = [(kt, ku, kv) for kt in kts for ku in kuo
                            for kv in range(3)]
                    ps0 = pp.tile([128, 6, HR, KW], fp32, bufs=2)
                    ps1 = pp.tile([128, 6, HR, KW], fp32, bufs=2)
                    ps2 = pp.tile([128, 6, HR, KW], fp32, bufs=1)
                    ps3 = pp.tile([128, 6, HR, KW], fp32, bufs=1)
                    ps4 = pp.tile([128, 6, HR, KW], fp32, bufs=1)
                    ps5 = pp.tile([128, 6, HR, KW], fp32, bufs=1)
                    prs = ((ps0, ps1), (ps2, ps3), (ps4, ps5))
                    ph = stp.tile([128, 4, 6, HR, KW + 2], fp16)
                    sa = stp.tile([128, 6, HR, KW], fp16)
                    sb = stp.tile([128, 6, HR, KW], fp16)
                    sc = stp.tile([128, 6, HR, KW], fp16)
                    sd = stp.tile([128, 6, HR, KW], fp16)
                    se = stp.tile([128, 6, HR, KW], fp16)
                    t2 = stp.tile([128, 6, HR, KW], fp16)
                    t4 = stp.tile([128, 6, HR, KW], fp16)

                    for p in range(3):
                        pa, pb = prs[p]
                        for i, (kt, ku, kv) in enumerate(taps):
                            tp = t + kt - 1
                            dp = dlo + ku
                            g = _g27(kt, ku, kv)
                            st_f = (i == 0)
                            sp_f = (i == len(taps) - 1)
                            r0 = 1 if (dlo == 0 and ku == 0) else 0
                            r1 = nd - (1 if (dlo + nd == D and ku == 2) else 0)
                            nc.tensor.matmul(
                                pa[:, r0:r1, :, :], w1[0:64, p, g, :],
                                xPs[p][0:64, tp, dp + r0:dp + r1,
                                       kv:kv + HR, :],
                                start=st_f, stop=sp_f, tile_position=(0, 0))
                            nc.tensor.matmul(
                                pb[:, r0:r1, :, :], w1[64:128, p, g, :],
                                xPs[p][64:128, tp, dp + r0:dp + r1,
                                       kv:kv + HR, :],
                                start=st_f, stop=sp_f, tile_position=(64, 0))
                        if p == 0:
                            # a=m1+m2, b=m1-m2 (<=1 PSUM input per DVE op)
                            nc.scalar.activation(t2[:, 0:nd, :, :],
                                                 ps1[:, 0:nd, :, :], IDENT)
                            nc.vector.tensor_add(sa[:, 0:nd, :, :],
                                                 ps0[:, 0:nd, :, :],
                                                 t2[:, 0:nd, :, :])
                            nc.vector.tensor_sub(sb[:, 0:nd, :, :],
                                                 ps0[:, 0:nd, :, :],
                                                 t2[:, 0:nd, :, :])
                        elif p == 1:
                            # c=m3+m4, d=m3-m4, e=a+c; y1=2d+b y2=4c+a y3p=8d+b
                            nc.scalar.activation(t4[:, 0:nd, :, :],
                                                 ps3[:, 0:nd, :, :], IDENT)
                            nc.vector.tensor_add(sc[:, 0:nd, :, :],
                                                 ps2[:, 0:nd, :, :],
                                                 t4[:, 0:nd, :, :])
                            nc.vector.tensor_sub(sd[:, 0:nd, :, :],
                                                 ps2[:, 0:nd, :, :],
                                                 t4[:, 0:nd, :, :])
                            nc.vector.tensor_add(se[:, 0:nd, :, :],
                                                 sa[:, 0:nd, :, :],
                                                 sc[:, 0:nd, :, :])
                            nc.vector.scalar_tensor_tensor(
                                ph[:, 1, 0:nd, :, 1:9], sd[:, 0:nd, :, :], 2.0,
                                sb[:, 0:nd, :, :], MULT, ADD)
                            nc.vector.scalar_tensor_tensor(
                                ph[:, 2, 0:nd, :, 1:9], sc[:, 0:nd, :, :], 4.0,
                                sa[:, 0:nd, :, :], MULT, ADD)
                            nc.vector.scalar_tensor_tensor(
                                ph[:, 3, 0:nd, :, 1:9], sd[:, 0:nd, :, :], 8.0,
                                sb[:, 0:nd, :, :], MULT, ADD)
                        else:
                            # y0=m0+e, y3=y3p+m5
                            nc.vector.tensor_add(ph[:, 0, 0:nd, :, 1:9],
                                                 ps4[:, 0:nd, :, :],
                                                 se[:, 0:nd, :, :])
                            nc.vector.tensor_add(ph[:, 3, 0:nd, :, 1:9],
                                                 ph[:, 3, 0:nd, :, 1:9],
                                                 ps5[:, 0:nd, :, :])
                    # ReLU + b1 in place; p1/p3 first so the h~ chain can
                    # start after two relus (halo masking moves to hT below —
                    # the W-transform is row-linear so it commutes)
                    for p4 in (1, 3, 0, 2):
                        nc.scalar.activation(ph[:, p4, 0:nd, :, 1:9],
                                             ph[:, p4, 0:nd, :, 1:9],
                                             RELU, bias=b1[:, 0:1])
                    # h~ forward F(4,3) transform along W from phase planes
                    # window elems: h0=y3[k-1] h1=y0 h2=y1 h3=y2 h4=y3 h5=y0[k+1]
                    y0c = ph[:, 0, 0:nd, :, 1:9]
                    y1c = ph[:, 1, 0:nd, :, 1:9]
                    y2c = ph[:, 2, 0:nd, :, 1:9]
                    y3c = ph[:, 3, 0:nd, :, 1:9]
                    y3l = ph[:, 3, 0:nd, :, 0:8]
                    y0r = ph[:, 0, 0:nd, :, 2:10]
                    u_ = stp.tile([128, 6, HR, KW], fp16)
                    v_ = stp.tile([128, 6, HR, KW], fp16)
                    wz = stp.tile([128, 6, HR, KW], fp16)
                    zz = stp.tile([128, 6, HR, KW], fp16)
                    qq = stp.tile([128, 6, HR, KW], fp16)
                    d0, d1 = 1 + dlo, 1 + dlo + nd
                    nc.vector.scalar_tensor_tensor(
                        u_[:, 0:nd, :, :], y1c, -4.0, y3c, MULT, ADD)
                    nc.vector.tensor_sub(wz[:, 0:nd, :, :], y3c, y1c)
                    nc.vector.scalar_tensor_tensor(
                        qq[:, 0:nd, :, :], y1c, -5.0, y3c, MULT, ADD)
                    nc.vector.scalar_tensor_tensor(
                        hT[:, t, 0, d0:d1, :, :], y3l, 4.0,
                        qq[:, 0:nd, :, :], MULT, ADD)
                    nc.vector.scalar_tensor_tensor(
                        v_[:, 0:nd, :, :], y0c, -4.0, y2c, MULT, ADD)
                    nc.vector.tensor_sub(zz[:, 0:nd, :, :], y2c, y0c)
                    nc.vector.tensor_add(hT[:, t, 1, d0:d1, :, :],
                                         u_[:, 0:nd, :, :], v_[:, 0:nd, :, :])
                    nc.vector.tensor_sub(hT[:, t, 2, d0:d1, :, :],
                                         u_[:, 0:nd, :, :], v_[:, 0:nd, :, :])
                    nc.vector.scalar_tensor_tensor(
                        hT[:, t, 3, d0:d1, :, :], zz[:, 0:nd, :, :], 2.0,
                        wz[:, 0:nd, :, :], MULT, ADD)
                    nc.vector.scalar_tensor_tensor(
                        hT[:, t, 4, d0:d1, :, :], zz[:, 0:nd, :, :], -2.0,
                        wz[:, 0:nd, :, :], MULT, ADD)
                    nc.vector.scalar_tensor_tensor(
                        qq[:, 0:nd, :, :], y2c, -5.0, y0r, MULT, ADD)
                    nc.vector.scalar_tensor_tensor(
                        hT[:, t, 5, d0:d1, :, :], y0c, 4.0,
                        qq[:, 0:nd, :, :], MULT, ADD)
                    # zero out-of-image halo rows of h~ (edge cores only)
                    nc.vector.tensor_scalar_mul(
                        hT[:, t, :, d0:d1, 0, :], hT[:, t, :, d0:d1, 0, :],
                        mt[:, 0:1])
                    nc.vector.tensor_scalar_mul(
                        hT[:, t, :, d0:d1, HR - 1, :],
                        hT[:, t, :, d0:d1, HR - 1, :], mb[:, 0:1])

            # ---- conv2: F(4,3) comps, K=128 col-split co-stream ----
            def conv2_t(t, runs):
                kts = _t_taps(t)
                for dlo, nd in runs:
                    n = nd * SH * KW   # 512
                    kuo = (1, 2, 0) if dlo == 0 else (0, 1, 2)
                    taps = [(kt, ku, kv) for kt in kts for ku in kuo
                            for kv in range(3)]
                    ps0 = pp.tile([128, 512], fp32, bufs=2)
                    ps1 = pp.tile([128, 512], fp32, bufs=2)
                    ps2 = pp.tile([128, 512], fp32, bufs=1)
                    qf = (ps0, ps1, ps2)
                    c2a = stp.tile([64, 512], fp16)
                    c2b = stp.tile([64, 512], fp16)
                    c2c = stp.tile([64, 512], fp16)
                    c2d = stp.tile([64, 512], fp16)
                    c2e = stp.tile([64, 512], fp16)
                    c2t2 = stp.tile([64, 512], fp16)
                    c2t4 = stp.tile([64, 512], fp16)
                    yst = stp.tile([64, 4, 512], fp16)
                    for p, (ca, cb) in enumerate(PAIRS):
                        for i, (kt, ku, kv) in enumerate(taps):
                            tp = t + kt - 1
                            dp = dlo + ku
                            g = _g27(kt, ku, kv)
                            st_f = (i == 0)
                            sp_f = (i == len(taps) - 1)
                            r0 = 1 if (dlo == 0 and ku == 0) else 0
                            r1 = nd - (1 if (dlo + nd == D and ku == 2) else 0)
                            nc.tensor.matmul(
                                qf[p][0:64, r0 * 64:r1 * 64], w2[:, g, ca, :],
                                hT[:, tp, ca, dp + r0:dp + r1, kv:kv + SH, :],
                                start=st_f, stop=sp_f, tile_position=(0, 0))
                            nc.tensor.matmul(
                                qf[p][64:128, r0 * 64:r1 * 64], w2[:, g, cb, :],
                                hT[:, tp, cb, dp + r0:dp + r1, kv:kv + SH, :],
                                start=st_f, stop=sp_f, tile_position=(0, 64))
                        if p == 0:
                            # fold b2 into a and b: every y phase then carries
                            # the bias (y0=m0+e, y1=2d+b, y2=4c+a, y3=8d+b+m5)
                            nc.scalar.activation(c2t2[:, 0:n],
                                                 qf[0][64:128, 0:n], IDENT)
                            nc.vector.scalar_tensor_tensor(
                                c2a[:, 0:n], qf[0][0:64, 0:n], b2[:, 0:1],
                                c2t2[:, 0:n], ADD, ADD)
                            nc.vector.scalar_tensor_tensor(
                                c2b[:, 0:n], qf[0][0:64, 0:n], b2[:, 0:1],
                                c2t2[:, 0:n], ADD, SUB)
                        elif p == 1:
                            nc.scalar.activation(c2t4[:, 0:n],
                                                 qf[1][64:128, 0:n], IDENT)
                            nc.vector.tensor_add(c2c[:, 0:n], qf[1][0:64, 0:n],
                                                 c2t4[:, 0:n])
                            nc.vector.tensor_sub(c2d[:, 0:n], qf[1][0:64, 0:n],
                                                 c2t4[:, 0:n])
                            nc.vector.tensor_add(c2e[:, 0:n], c2a[:, 0:n],
                                                 c2c[:, 0:n])
                            nc.vector.scalar_tensor_tensor(
                                yst[:, 1, 0:n], c2d[:, 0:n], 2.0,
                                c2b[:, 0:n], MULT, ADD)
                            nc.vector.scalar_tensor_tensor(
                                yst[:, 2, 0:n], c2c[:, 0:n], 4.0,
                                c2a[:, 0:n], MULT, ADD)
                            nc.vector.scalar_tensor_tensor(
                                yst[:, 3, 0:n], c2d[:, 0:n], 8.0,
                                c2b[:, 0:n], MULT, ADD)
                        else:
                            nc.vector.tensor_add(yst[:, 0, 0:n],
                                                 qf[2][0:64, 0:n],
                                                 c2e[:, 0:n])
                            nc.vector.tensor_add(yst[:, 3, 0:n],
                                                 yst[:, 3, 0:n],
                                                 qf[2][64:128, 0:n])
                    base = dlo * SH * KW
                    nc.sync.dma_start(y_d.ap()[:, t, 1:3, base:base + n],
                                      yst[:, 1:3, 0:n])
                    nc.sync.dma_start(y_d.ap()[:, t, 0:1, base:base + n],
                                      yst[:, 0:1, 0:n])
                    nc.sync.dma_start(y_d.ap()[:, t, 3:4, base:base + n],
                                      yst[:, 3:4, 0:n])

            # interleave at t granularity: conv2 t depends on conv1 t-1..t+1;
            # one full conv1 t-block of distance keeps the PE queue from
            # head-of-line blocking on fresh h~ DVE results
            conv1_t(0, RUNS1)
            conv1_t(1, RUNS1)
            conv1_t(2, RUNS1[0:1])
            conv2_t(0, RUNS2)
            conv1_t(2, RUNS1[1:3])
            conv1_t(3, RUNS1[0:1])
            conv2_t(1, RUNS2)
            conv1_t(3, RUNS1[1:3])
            conv2_t(2, RUNS2)
            conv2_t(3, RUNS2)
    nc.compile()
    return nc


def kernel(x, w1, b1, w2, b2):
    from concourse.bass_utils import run_bass_kernel_spmd

    hostd = _make_host_arrays(x, w1, b1, w2, b2)
    if "nc" not in _cache:
        _cache["nc"] = _build_module()
    nc = _cache["nc"]

    in_maps = []
    for core in range(NCORES):
        in_maps.append({
            "xp0": hostd["XP"][core][0], "xp1": hostd["XP"][core][1],
            "xp2": hostd["XP"][core][2], "aux": hostd["AUX"][core],
            "w1": hostd["W1"], "w2": hostd["W2"],
        })
    res = run_bass_kernel_spmd(nc, in_maps, core_ids=list(range(NCORES)))

    y = np.zeros((B, C_OUT, T, D, H, W), np.float32)
    for core in range(NCORES):
        b, j = divmod(core, NJ)
        yc = res.results[core]["y"].reshape(C_OUT, T, 4, D, SH, KW)
        ys = y[b, :, :, :, SH * j:SH * (j + 1), :]
        for p in range(4):
            ys[..., p::4] = yc[:, :, p]
    return y


# revision 4
# speedup vs baseline: 1.0105x; 1.0105x over previous
"""4D Conv-MLP (conv3^4 -> ReLU -> conv3^4) on 8 Trainium2 NeuronCores.

Sharding: core = b*4 + j (batch b in {0,1}, H-slab j in {0..3}, 8 output rows
each). Conv1 is recomputed on a 1-row h halo (10 h rows from 12 x rows), so no
cross-core communication. One SPMD program; boundary behavior is data-driven
(host-zeroed x halos + h halo-row masks).

Winograd F(4,3) along W on BOTH convs (2x fewer multiplies vs direct): the 3
kw taps become 6 pointwise components m0..m5 evaluated at 8 stride-4 w-tiles;
y = A^T m (4 outputs per tile). T/D/H taps (kt, ku, kv) stay direct.

  - x~ (input transform, 6 comps) built on host; comps packed in PAIRS
    (1,2),(3,4),(0,5) on the 128 partitions (64 ch each half) so two K=64
    matmuls co-stream via tile_position (0,0)/(64,0).
  - d runs are uniform via ZERO-PADDED d planes (18 = 1+16+1): conv1 runs
    nd=6,6,4 (N<=480), conv2 nd=8,8 (N=512); all 3 ku taps always valid
    (pad-plane taps multiply zeros).
  - conv1 PSUM: 6 chains/run, one bank each; pair (1,2) double-buffered,
    (3,4)/(0,5) single (8 banks exactly). Inverse transform pipelines at
    pair granularity: a=m1+m2, b=m1-m2 | c=m3+m4, d=m3-m4, e=a+c,
    y1=2d+b, y2=4c+a, y3p=8d+b | y0=m0+e, y3=y3p+m5 (scalar_tensor_tensor
    fused ops); ReLU+b1 in-place on Scalar; halo rows masked; h~ (conv2's
    F(4,3) forward transform) built with 12 fused DVE ops from the 4
    phase planes (pad cols give the k+-1 shifts).
  - conv2: K=128 matmuls, comps col-split co-streamed via tile_position
    (0,0)/(0,64) into one bank per pair; same 10-op inverse; bias via
    Scalar; one DMA per run to y (4 phase planes, host interleaves).
All matmul operands fp16, PSUM accumulation fp32. PE is stream-bound:
~595k column-cycles @2.4GHz ~ 248us.
"""

import numpy as np

B, C_IN, C_HID, C_OUT = 2, 64, 128, 64
T, D, H, W = 4, 16, 32, 32
NCORES, NJ = 8, 4
SH = H // NJ          # 8 out rows per slab
XH = SH + 4           # 12 x rows per slab
HR = SH + 2           # 10 h rows per slab (1-row halo each side)
KW = W // 4           # 8 winograd w-tiles
DP = D + 2            # zero-padded d planes
XPL = D * XH * KW     # x~ real planes per t per comp-pair = 1536
PAIRS = ((1, 2), (3, 4), (0, 5))
RUNS1 = ((0, 6), (6, 6), (12, 4))
RUNS2 = ((0, 8), (8, 8))

# F(4,3) transform matrices (float64 host math)
_BT = np.array([[4, 0, -5, 0, 1, 0], [0, -4, -4, 1, 1, 0], [0, 4, -4, -1, 1, 0],
                [0, -2, -1, 2, 1, 0], [0, 2, -1, -2, 1, 0], [0, 4, 0, -5, 0, 1]],
               np.float64)
_G = np.array([[1 / 4, 0, 0], [-1 / 6, -1 / 6, -1 / 6], [-1 / 6, 1 / 6, -1 / 6],
               [1 / 24, 1 / 12, 1 / 6], [1 / 24, -1 / 12, 1 / 6], [0, 0, 1]],
              np.float64)

_cache = {}


def _t_taps(t):
    return [kt for kt in range(3) if 0 <= t + kt - 1 < T]


def _g27(kt, ku, kv):
    return (kt * 3 + ku) * 3 + kv


def _make_host_arrays(x, w1, b1, w2, b2):
    x = np.asarray(x, np.float32)
    XPs, AUXs = [], []
    for core in range(NCORES):
        b, j = divmod(core, NJ)
        h0 = SH * j
        slab = np.zeros((C_IN, T, D, XH, W + 6), np.float32)
        lo, hi = h0 - 2, h0 + 10
        slo, shi = max(lo, 0), min(hi, H)
        slab[:, :, :, slo - lo:shi - lo, 1:33] = x[b, :, :, :, slo:shi, :]
        # F(4,3) input transform along W: tile k reads padded cols 4k..4k+5
        win = np.stack([slab[..., 4 * k:4 * k + 6] for k in range(KW)], -2)
        xt = (win @ _BT.T).astype(np.float16)    # [64, T, D, XH, KW, 6]
        ps = []
        for ca, cb in PAIRS:
            ps.append(np.concatenate([xt[..., ca], xt[..., cb]], 0)
                      .reshape(128, T, XPL))
        XPs.append(ps)
        a = np.zeros((128, 4), np.float32)
        a[:, 0] = np.asarray(b1, np.float32)
        a[:, 1] = 0.0 if j == 0 else 1.0
        a[:, 2] = 0.0 if j == NJ - 1 else 1.0
        a[0:64, 3] = np.asarray(b2, np.float32)
        AUXs.append(a)

    w1 = np.asarray(w1, np.float64)   # [128, 64, 3,3,3,3]
    w2 = np.asarray(w2, np.float64)   # [64, 128, 3,3,3,3]
    w1t = np.einsum('oitduv,cv->oitduc', w1, _G)   # [128, 64, 3,3,3, 6]
    w2t = np.einsum('oitduv,cv->oitduc', w2, _G)   # [64, 128, 3,3,3, 6]
    W1P = np.zeros((128, 3, 27, 128), np.float16)
    W2P = np.zeros((128, 27, 6, 64), np.float16)
    for kt in range(3):
        for ku in range(3):
            for kv in range(3):
                g = _g27(kt, ku, kv)
                for p, (ca, cb) in enumerate(PAIRS):
                    W1P[0:64, p, g, :] = w1t[:, :, kt, ku, kv, ca].T
                    W1P[64:128, p, g, :] = w1t[:, :, kt, ku, kv, cb].T
                for c in range(6):
                    W2P[:, g, c, :] = w2t[:, :, kt, ku, kv, c].T
    return dict(XP=XPs, AUX=AUXs,
                W1=W1P.reshape(128, 3 * 27 * 128), W2=W2P.reshape(128, 27 * 6 * 64))


def _build_module():
    import concourse.tile as tile
    from concourse import bacc, mybir

    fp16 = mybir.dt.float16
    fp32 = mybir.dt.float32
    RELU = mybir.ActivationFunctionType.Relu
    IDENT = mybir.ActivationFunctionType.Identity
    MULT = mybir.AluOpType.mult
    ADD = mybir.AluOpType.add
    SUB = mybir.AluOpType.subtract

    nc = bacc.Bacc("TRN2", target_bir_lowering=False, debug=False, num_devices=1)
    xp_d = [nc.dram_tensor(f"xp{p}", [128, T, XPL], fp16, kind="ExternalInput")
            for p in range(3)]
    w1_d = nc.dram_tensor("w1", [128, 3 * 27 * 128], fp16, kind="ExternalInput")
    w2_d = nc.dram_tensor("w2", [128, 27 * 6 * 64], fp16, kind="ExternalInput")
    # aux: col0=b1, col1=mt, col2=mb, col3[:64]=b2
    aux_d = nc.dram_tensor("aux", [128, 4], fp32, kind="ExternalInput")
    # 4 phase planes (w%4) stored separately; host interleaves
    y_d = nc.dram_tensor("y", [64, T, 4, D * SH * KW], fp16,
                         kind="ExternalOutput")

    with tile.TileContext(nc) as tc:
        with (
            tc.tile_pool(name="xw", bufs=1) as xw,
            tc.tile_pool(name="st", bufs=2) as stp,
            tc.tile_pool(name="pp", bufs=1, space="PSUM") as pp,
        ):
            xP0 = xw.tile([128, T, DP, XH, KW], fp16)
            xP1 = xw.tile([128, T, DP, XH, KW], fp16)
            xP2 = xw.tile([128, T, DP, XH, KW], fp16)
            xPs = (xP0, xP1, xP2)
            w1 = xw.tile([128, 3, 27, 128], fp16)
            w2 = xw.tile([128, 27, 6, 64], fp16)
            hT = xw.tile([128, T, 6, DP, HR, KW], fp16)
            aux = xw.tile([128, 4], fp32)

            # zero the d pad planes (never DMA'd / written by conv1)
            for xp in xPs:
                nc.vector.memset(xp[:, :, 0, :, :], 0.0)
                nc.vector.memset(xp[:, :, DP - 1, :, :], 0.0)
            nc.vector.memset(hT[:, :, :, 0, :, :], 0.0)
            nc.vector.memset(hT[:, :, :, DP - 1, :, :], 0.0)

            # DMA order: unblock conv1 t=0 run(0,6) fast — its tap order
            # is g 12..17, 9..11, 21..26, 18..20 on d planes 0..7. Input
            # transfers alternate between the two HW DGE queues (qSP via
            # nc.sync, qAct via nc.scalar) — each queue drains serially.
            def idma(dst, srcv):
                nc.sync.dma_start(dst, srcv)

            def w1chunk(p, glo, ghi):
                idma(w1[:, p, glo:ghi, :],
                     w1_d.ap()[:, p * 3456 + glo * 128:p * 3456 + ghi * 128])

            nc.gpsimd.dma_start(xPs[0][:, 0, 1:9, :, :],
                                xp_d[0].ap()[:, 0, 0:768])
            nc.gpsimd.dma_start(xPs[0][:, 1, 1:9, :, :],
                                xp_d[0].ap()[:, 1, 0:768])
            for p in range(3):
                if p == 0:
                    w1chunk(0, 12, 15)
                    w1chunk(0, 15, 18)
                    w1chunk(0, 9, 12)
                    w1chunk(0, 18, 27)
                else:
                    w1chunk(p, 9, 27)
                    for t in (0, 1):
                        idma(xPs[p][:, t, 1:9, :, :], xp_d[p].ap()[:, t, 0:768])
            idma(aux[:, :], aux_d.ap())
            for p in range(3):
                for t in (0, 1):
                    idma(xPs[p][:, t, 9:DP - 1, :, :],
                         xp_d[p].ap()[:, t, 768:1536])
                w1chunk(p, 0, 9)
            for p in range(3):
                for t in (2, 3):
                    idma(xPs[p][:, t, 1:DP - 1, :, :], xp_d[p].ap()[:, t, :])
            idma(w2[:, :, :, :], w2_d.ap())
            b1 = aux[:, 0:1]
            mt = aux[:, 1:2]
            mb = aux[:, 2:3]
            b2 = aux[0:64, 3:4]

            # prime the phase-plane staging pads (cols 0 and 9 stay zero)
            for _ in range(2):
                ph = stp.tile([128, 4, 6, HR, KW + 2], fp16)
                nc.vector.memset(ph[:, :, :, :, :], 0.0)

            # ---- conv1: F(4,3) comps, K=64 row-split co-stream ----
            def conv1_t(t, runs):
                kts = _t_taps(t)
                for dlo, nd in runs:
                    # pad-plane taps stream partial N (trimmed rows); first
                    # tap must be untrimmed so start=True clears the full bank
                    kuo = (1, 2, 0) if dlo == 0 else (0, 1, 2)
                    taps = [(kt, ku, kv) for kt in kts for ku in kuo
                            for kv in range(3)]
                    ps0 = pp.tile([128, 6, HR, KW], fp32, bufs=2)
                    ps1 = pp.tile([128, 6, HR, KW], fp32, bufs=2)
                    ps2 = pp.tile([128, 6, HR, KW], fp32, bufs=1)
                    ps3 = pp.tile([128, 6, HR, KW], fp32, bufs=1)
                    ps4 = pp.tile([128, 6, HR, KW], fp32, bufs=1)
                    ps5 = pp.tile([128, 6, HR, KW], fp32, bufs=1)
                    prs = ((ps0, ps1), (ps2, ps3), (ps4, ps5))
                    ph = stp.tile([128, 4, 6, HR, KW + 2], fp16)
                    sa = stp.tile([128, 6, HR, KW], fp16)
                    sb = stp.tile([128, 6, HR, KW], fp16)
                    sc = stp.tile([128, 6, HR, KW], fp16)
                    sd = stp.tile([128, 6, HR, KW], fp16)
                    se = stp.tile([128, 6, HR, KW], fp16)
                    t2 = stp.tile([128, 6, HR, KW], fp16)
                    t4 = stp.tile([128, 6, HR, KW], fp16)

                    for p in range(3):
                        pa, pb = prs[p]
                        for i, (kt, ku, kv) in enumerate(taps):
                            tp = t + kt - 1
                            dp = dlo + ku
                            g = _g27(kt, ku, kv)
                            st_f = (i == 0)
                            sp_f = (i == len(taps) - 1)
                            r0 = 1 if (dlo == 0 and ku == 0) else 0
                            r1 = nd - (1 if (dlo + nd == D and ku == 2) else 0)
                            nc.tensor.matmul(
                                pa[:, r0:r1, :, :], w1[0:64, p, g, :],
                                xPs[p][0:64, tp, dp + r0:dp + r1,
                                       kv:kv + HR, :],
                                start=st_f, stop=sp_f, tile_position=(0, 0))
                            nc.tensor.matmul(
                                pb[:, r0:r1, :, :], w1[64:128, p, g, :],
                                xPs[p][64:128, tp, dp + r0:dp + r1,
                                       kv:kv + HR, :],
                                start=st_f, stop=sp_f, tile_position=(64, 0))
                        if p == 0:
                            # a=m1+m2, b=m1-m2 (<=1 PSUM input per DVE op)
                            nc.scalar.activation(t2[:, 0:nd, :, :],
                                                 ps1[:, 0:nd, :, :], IDENT)
                            nc.vector.tensor_add(sa[:, 0:nd, :, :],
                                                 ps0[:, 0:nd, :, :],
                                                 t2[:, 0:nd, :, :])
                            nc.vector.tensor_sub(sb[:, 0:nd, :, :],
                                                 ps0[:, 0:nd, :, :],
                                                 t2[:, 0:nd, :, :])
                        elif p == 1:
                            # c=m3+m4, d=m3-m4, e=a+c; y1=2d+b y2=4c+a y3p=8d+b
                            nc.scalar.activation(t4[:, 0:nd, :, :],
                                                 ps3[:, 0:nd, :, :], IDENT)
                            nc.vector.tensor_add(sc[:, 0:nd, :, :],
                                                 ps2[:, 0:nd, :, :],
                                                 t4[:, 0:nd, :, :])
                            nc.vector.tensor_sub(sd[:, 0:nd, :, :],
                                                 ps2[:, 0:nd, :, :],
                                                 t4[:, 0:nd, :, :])
                            nc.vector.tensor_add(se[:, 0:nd, :, :],
                                                 sa[:, 0:nd, :, :],
                                                 sc[:, 0:nd, :, :])
                            nc.vector.scalar_tensor_tensor(
                                ph[:, 1, 0:nd, :, 1:9], sd[:, 0:nd, :, :], 2.0,
                                sb[:, 0:nd, :, :], MULT, ADD)
                            nc.vector.scalar_tensor_tensor(
                                ph[:, 2, 0:nd, :, 1:9], sc[:, 0:nd, :, :], 4.0,
                                sa[:, 0:nd, :, :], MULT, ADD)
                            nc.vector.scalar_tensor_tensor(
                                ph[:, 3, 0:nd, :, 1:9], sd[:, 0:nd, :, :], 8.0,
                                sb[:, 0:nd, :, :], MULT, ADD)
                        else:
                            # y0=m0+e, y3=y3p+m5
                            nc.vector.tensor_add(ph[:, 0, 0:nd, :, 1:9],
                                                 ps4[:, 0:nd, :, :],
                                                 se[:, 0:nd, :, :])
                            nc.vector.tensor_add(ph[:, 3, 0:nd, :, 1:9],
                                                 ph[:, 3, 0:nd, :, 1:9],
                                                 ps5[:, 0:nd, :, :])
                    # ReLU + b1 in place; p1/p3 first so the h~ chain can
                    # start after two relus (halo masking moves to hT below —
                    # the W-transform is row-linear so it commutes)
                    for p4 in (1, 3, 0, 2):
                        nc.scalar.activation(ph[:, p4, 0:nd, :, 1:9],
                                             ph[:, p4, 0:nd, :, 1:9],
                                             RELU, bias=b1[:, 0:1])
                    # h~ forward F(4,3) transform along W from phase planes
                    # window elems: h0=y3[k-1] h1=y0 h2=y1 h3=y2 h4=y3 h5=y0[k+1]
                    y0c = ph[:, 0, 0:nd, :, 1:9]
                    y1c = ph[:, 1, 0:nd, :, 1:9]
                    y2c = ph[:, 2, 0:nd, :, 1:9]
                    y3c = ph[:, 3, 0:nd, :, 1:9]
                    y3l = ph[:, 3, 0:nd, :, 0:8]
                    y0r = ph[:, 0, 0:nd, :, 2:10]
                    u_ = stp.tile([128, 6, HR, KW], fp16)
                    v_ = stp.tile([128, 6, HR, KW], fp16)
                    wz = stp.tile([128, 6, HR, KW], fp16)
                    zz = stp.tile([128, 6, HR, KW], fp16)
                    qq = stp.tile([128, 6, HR, KW], fp16)
                    d0, d1 = 1 + dlo, 1 + dlo + nd
                    nc.vector.scalar_tensor_tensor(
                        u_[:, 0:nd, :, :], y1c, -4.0, y3c, MULT, ADD)
                    nc.vector.tensor_sub(wz[:, 0:nd, :, :], y3c, y1c)
                    nc.vector.scalar_tensor_tensor(
                        qq[:, 0:nd, :, :], y1c, -5.0, y3c, MULT, ADD)
                    nc.vector.scalar_tensor_tensor(
                        hT[:, t, 0, d0:d1, :, :], y3l, 4.0,
                        qq[:, 0:nd, :, :], MULT, ADD)
                    nc.vector.scalar_tensor_tensor(
                        v_[:, 0:nd, :, :], y0c, -4.0, y2c, MULT, ADD)
                    nc.vector.tensor_sub(zz[:, 0:nd, :, :], y2c, y0c)
                    nc.vector.tensor_add(hT[:, t, 1, d0:d1, :, :],
                                         u_[:, 0:nd, :, :], v_[:, 0:nd, :, :])
                    nc.vector.tensor_sub(hT[:, t, 2, d0:d1, :, :],
                                         u_[:, 0:nd, :, :], v_[:, 0:nd, :, :])
                    nc.vector.scalar_tensor_tensor(
                        hT[:, t, 3, d0:d1, :, :], zz[:, 0:nd, :, :], 2.0,
                        wz[:, 0:nd, :, :], MULT, ADD)
                    nc.vector.scalar_tensor_tensor(
                        hT[:, t, 4, d0:d1, :, :], zz[:, 0:nd, :, :], -2.0,
                        wz[:, 0:nd, :, :], MULT, ADD)
                    nc.vector.scalar_tensor_tensor(
                        qq[:, 0:nd, :, :], y2c, -5.0, y0r, MULT, ADD)
                    nc.vector.scalar_tensor_tensor(
                        hT[:, t, 5, d0:d1, :, :], y0c, 4.0,
                        qq[:, 0:nd, :, :], MULT, ADD)
                    # zero out-of-image halo rows of h~ (edge cores only)
                    nc.vector.tensor_scalar_mul(
                        hT[:, t, :, d0:d1, 0, :], hT[:, t, :, d0:d1, 0, :],
                        mt[:, 0:1])
                    nc.vector.tensor_scalar_mul(
                        hT[:, t, :, d0:d1, HR - 1, :],
                        hT[:, t, :, d0:d1, HR - 1, :], mb[:, 0:1])

            # ---- conv2: F(4,3) comps, K=128 col-split co-stream ----
            def conv2_t(t, runs):
                kts = _t_taps(t)
                for dlo, nd in runs:
                    n = nd * SH * KW   # 512
                    kuo = (1, 2, 0) if dlo == 0 else (0, 1, 2)
                    taps = [(kt, ku, kv) for kt in kts for ku in kuo
                            for kv in range(3)]
                    ps0 = pp.tile([128, 512], fp32, bufs=2)
                    ps1 = pp.tile([128, 512], fp32, bufs=2)
                    ps2 = pp.tile([128, 512], fp32, bufs=1)
                    qf = (ps0, ps1, ps2)
                    c2a = stp.tile([64, 512], fp16)
                    c2b = stp.tile([64, 512], fp16)
                    c2c = stp.tile([64, 512], fp16)
                    c2d = stp.tile([64, 512], fp16)
                    c2e = stp.tile([64, 512], fp16)
                    c2t2 = stp.tile([64, 512], fp16)
                    c2t4 = stp.tile([64, 512], fp16)
                    yst = stp.tile([64, 4, 512], fp16)
                    for p, (ca, cb) in enumerate(PAIRS):
                        for i, (kt, ku, kv) in enumerate(taps):
                            tp = t + kt - 1
                            dp = dlo + ku
                            g = _g27(kt, ku, kv)
                            st_f = (i == 0)
                            sp_f = (i == len(taps) - 1)
                            r0 = 1 if (dlo == 0 and ku == 0) else 0
                            r1 = nd - (1 if (dlo + nd == D and ku == 2) else 0)
                            nc.tensor.matmul(
                                qf[p][0:64, r0 * 64:r1 * 64], w2[:, g, ca, :],
                                hT[:, tp, ca, dp + r0:dp + r1, kv:kv + SH, :],
                                start=st_f, stop=sp_f, tile_position=(0, 0))
                            nc.tensor.matmul(
                                qf[p][64:128, r0 * 64:r1 * 64], w2[:, g, cb, :],
                                hT[:, tp, cb, dp + r0:dp + r1, kv:kv + SH, :],
                                start=st_f, stop=sp_f, tile_position=(0, 64))
                        if p == 0:
                            # fold b2 into a and b: every y phase then carries
                            # the bias (y0=m0+e, y1=2d+b, y2=4c+a, y3=8d+b+m5)
                            nc.scalar.activation(c2t2[:, 0:n],
                                                 qf[0][64:128, 0:n], IDENT)
                            nc.vector.scalar_tensor_tensor(
                                c2a[:, 0:n], qf[0][0:64, 0:n], b2[:, 0:1],
                                c2t2[:, 0:n], ADD, ADD)
                            nc.vector.scalar_tensor_tensor(
                                c2b[:, 0:n], qf[0][0:64, 0:n], b2[:, 0:1],
                                c2t2[:, 0:n], ADD, SUB)
                        elif p == 1:
                            nc.scalar.activation(c2t4[:, 0:n],
                                                 qf[1][64:128, 0:n], IDENT)
                            nc.vector.tensor_add(c2c[:, 0:n], qf[1][0:64, 0:n],
                                                 c2t4[:, 0:n])
                            nc.vector.tensor_sub(c2d[:, 0:n], qf[1][0:64, 0:n],
                                                 c2t4[:, 0:n])
                            nc.vector.tensor_add(c2e[:, 0:n], c2a[:, 0:n],
                                                 c2c[:, 0:n])
                            nc.vector.scalar_tensor_tensor(
                                yst[:, 1, 0:n], c2d[:, 0:n], 2.0,
                                c2b[:, 0:n], MULT, ADD)
                            nc.vector.scalar_tensor_tensor(
                                yst[:, 2, 0:n], c2c[:, 0:n], 4.0,
                                c2a[:, 0:n], MULT, ADD)
                            nc.vector.scalar_tensor_tensor(
                                yst[:, 3, 0:n], c2d[:, 0:n], 8.0,
                                c2b[:, 0:n], MULT, ADD)
                        else:
                            nc.vector.tensor_add(yst[:, 3, 0:n],
                                                 yst[:, 3, 0:n],
                                                 qf[2][64:128, 0:n])
                            nc.vector.tensor_add(yst[:, 0, 0:n],
                                                 qf[2][0:64, 0:n],
                                                 c2e[:, 0:n])
                    base = dlo * SH * KW
                    nc.sync.dma_start(y_d.ap()[:, t, 1:3, base:base + n],
                                      yst[:, 1:3, 0:n])
                    nc.sync.dma_start(y_d.ap()[:, t, 3:4, base:base + n],
                                      yst[:, 3:4, 0:n])
                    nc.sync.dma_start(y_d.ap()[:, t, 0:1, base:base + n],
                                      yst[:, 0:1, 0:n])

            # interleave at t granularity: conv2 t depends on conv1 t-1..t+1;
            # one full conv1 t-block of distance keeps the PE queue from
            # head-of-line blocking on fresh h~ DVE results
            conv1_t(0, RUNS1)
            conv1_t(1, RUNS1)
            conv1_t(2, RUNS1[0:1])
            conv2_t(0, RUNS2)
            conv1_t(2, RUNS1[1:3])
            conv1_t(3, RUNS1[0:1])
            conv2_t(1, RUNS2)
            conv1_t(3, RUNS1[1:3])
            conv2_t(2, RUNS2)
            conv2_t(3, RUNS2)
    nc.compile()
    return nc


def kernel(x, w1, b1, w2, b2):
    from concourse.bass_utils import run_bass_kernel_spmd

    hostd = _make_host_arrays(x, w1, b1, w2, b2)
    if "nc" not in _cache:
        _cache["nc"] = _build_module()
    nc = _cache["nc"]

    in_maps = []
    for core in range(NCORES):
        in_maps.append({
            "xp0": hostd["XP"][core][0], "xp1": hostd["XP"][core][1],
            "xp2": hostd["XP"][core][2], "aux": hostd["AUX"][core],
            "w1": hostd["W1"], "w2": hostd["W2"],
        })
    res = run_bass_kernel_spmd(nc, in_maps, core_ids=list(range(NCORES)))

    y = np.zeros((B, C_OUT, T, D, H, W), np.float32)
    for core in range(NCORES):
        b, j = divmod(core, NJ)
        yc = res.results[core]["y"].reshape(C_OUT, T, 4, D, SH, KW)
        ys = y[b, :, :, :, SH * j:SH * (j + 1), :]
        for p in range(4):
            ys[..., p::4] = yc[:, :, p]
    return y
